# revision 22
# baseline (speedup 1.0000x reference)
"""Trainium2 Bass kernel for nn_AwkwardRNN (4-layer LSTM, H2=2048, T=2048, batch-1).

Design ("layer-per-core wavefront pipeline", v2):
  - Cores 0-3 each own one LSTM layer. W_hh lives in SBUF as fp8 (pre-scaled
    by SCALE so values sit in e4m3's normal range); h is bf16; PSUM
    accumulates fp32; cell state c is fp32. Cores 4-7 run the same program
    on zero weights (outputs ignored).
  - The sequence runs in blocks of B=64 steps with a per-layer slot skew of
    2 (layer l processes block b in slot b+2l). The extra slot of skew gives
    the inter-layer AllGather a full slot of slack: AG(s) is issued at the
    end of slot s and consumed by the xw GEMM at the end of slot s+1, so the
    collective latency never stalls the PE.
  - Per step, W_hh @ h streams W as the matmul moving operand (h is a
    one-column stationary), 4-way column-tiled. The per-step xw[t] is
    injected as an extra K=UB one-hot matmul chunk.
  - xw blocks live in DRAM (ping-pong); the per-UB-iteration xw4 SBUF tile
    is double-buffered with a one-iteration prefetch distance inside the
    hardware loop (body covers 2 iterations = 8 steps), so the DMA latency
    is fully hidden.
  - h is re-laid-out for the next step's stationary with a DVE 32x32 block
    transpose; a host-side weight permutation absorbs the layout.
"""

import sys

for _p in ("/opt/trn_rl_repo",):
    if _p not in sys.path:
        sys.path.insert(0, _p)

from contextlib import ExitStack

import numpy as np
import ml_dtypes

import concourse.bacc as bacc
import concourse.bass as bass
import concourse.tile as tile
from concourse import mybir

F32 = mybir.dt.float32
BF16 = mybir.dt.bfloat16


class Cfg:
    def __init__(self, H2=2048, T=2048, B=64, UB=4, L=4, NCORES=8,
                 SCALE=1024.0, SKEW=2):
        self.H2, self.T, self.B, self.UB, self.L = H2, T, B, UB, L
        self.NCORES, self.SCALE, self.SKEW = NCORES, SCALE, SKEW
        self.G = 4 * H2
        self.S4 = H2 // 4           # hidden slice per col-group
        self.NF = H2 // 128         # stationary h chunks
        self.NBLK = T // B
        self.NSLOT = self.NBLK + SKEW * (L - 1)
        self.NIT = B // UB
        assert H2 % 128 == 0 and T % B == 0 and B % UB == 0 and UB % 2 == 0
        assert self.S4 % 32 == 0 and self.NF % 4 == 0 and self.NIT % 2 == 0

    @property
    def W_DT(self):
        return mybir.dt.float8e4

    @property
    def W_NP(self):
        return ml_dtypes.float8_e4m3


def perm_cols(cfg):
    """perm[fi, p] = hidden index held at (partition p, stationary chunk fi)."""
    fi = np.arange(cfg.NF)[:, None]
    p = np.arange(128)[None, :]
    return cfg.S4 * (p // 32) + 32 * fi + (p % 32)


def gate_order(cfg):
    """gidx[nt*S4 + q] = weight row of xw column (nt=(j*4+x), q)."""
    H2, S4 = cfg.H2, cfg.S4
    gidx = np.zeros(cfg.G, np.int64)
    for j in range(4):
        for x in range(4):
            nt = j * 4 + x
            gidx[nt * S4:(nt + 1) * S4] = x * H2 + S4 * j + np.arange(S4)
    return gidx


def _eye_rep(cfg):
    e = np.zeros((128, cfg.UB), ml_dtypes.bfloat16)
    for j in range(4):
        for u in range(cfg.UB):
            e[32 * j + u, u] = 1
    return e


def pack_rows(cfg, vec):
    """[G] gate-ordered vector -> [128, 4*S4] with row 32j = (j,*) slices."""
    out = np.zeros((128, 4 * cfg.S4), vec.dtype)
    for j in range(4):
        out[32 * j] = vec[4 * j * cfg.S4:(4 * j + 4) * cfg.S4]
    return out


def prep_core_inputs(cfg, core, event, w_ih0, w_ih, w_hh, b_ih, b_hh):
    H2, S4, NF, B, G = cfg.H2, cfg.S4, cfg.NF, cfg.B, cfg.G
    perm = perm_cols(cfg)
    gidx = gate_order(cfg)
    lay = core if core < cfg.L else None
    bf = ml_dtypes.bfloat16

    whh = np.zeros((128, NF, 4, 4, S4), cfg.W_NP)
    if lay is not None:
        W = (w_hh[lay] * cfg.SCALE).astype(np.float32)
        q = np.arange(S4)
        for kc in range(NF):
            Wc = W[:, perm[kc]]                     # [G, 128]
            for j in range(4):
                for x in range(4):
                    rows = x * H2 + S4 * j + q
                    whh[:, kc, j, x, :] = Wc[rows, :].T.astype(cfg.W_NP)

    # wih layout: [16(nt), 128(p), NF(fi), S4(q)]
    wih = np.zeros((16, 128, NF, S4), cfg.W_NP)
    if lay is not None and lay > 0:
        W = (w_ih[lay - 1] * cfg.SCALE).astype(np.float32)
        q = np.arange(S4)
        for fi in range(NF):
            Wc = W[:, perm[fi]]                     # [G, 128]
            for j in range(4):
                for x in range(4):
                    nt = j * 4 + x
                    rows = x * H2 + S4 * j + q
                    wih[nt, :, fi, :] = Wc[rows, :].T.astype(cfg.W_NP)

    g0 = np.zeros(G, np.float32)
    if lay == 0:
        g0 = (w_ih0[:, 0] * cfg.SCALE)[gidx]
    wih0 = pack_rows(cfg, g0).astype(bf)

    gb = np.zeros(G, np.float32)
    if lay is not None:
        gb = ((b_ih[lay] + b_hh[lay]) * cfg.SCALE)[gidx]
    bias = pack_rows(cfg, gb).astype(bf)

    ev = np.zeros((1, cfg.T), bf)
    if lay == 0:
        ev[0] = event.astype(bf)

    # gmask[:, i] = 1 if block (i - SKEW*l) is valid for this core's layer.
    # gemm for slot s (producing xw consumed in slot s+1) masks with column
    # s+1; the prologue gemm (s=-1) uses column 0.
    gmask = np.zeros((128, cfg.NSLOT + 1), np.float32)
    if lay is not None:
        for i in range(cfg.NSLOT + 1):
            b = i - cfg.SKEW * lay
            gmask[:, i] = 1.0 if 0 <= b < cfg.NBLK else 0.0

    sel = np.zeros((128, 4), np.float32)
    if lay is not None and lay > 0:
        sel[:, lay - 1] = 1.0

    return {
        "whh": whh, "wih": wih, "wih0": wih0, "bias": bias, "ev": ev,
        "vnext": gmask, "sel": sel,
        "eye": _eye_rep(cfg),
        "ones": np.ones((128, B), bf),
    }


def build(cfg):
    H2, S4, NF, B, G, UB = cfg.H2, cfg.S4, cfg.NF, cfg.B, cfg.G, cfg.UB
    f8 = cfg.W_DT
    Sig = mybir.ActivationFunctionType.Sigmoid
    Tanh = mybir.ActivationFunctionType.Tanh
    inv = 1.0 / cfg.SCALE
    NFQ = NF // 4

    nc = bacc.Bacc("TRN2", target_bir_lowering=False)

    d_whh = nc.dram_tensor("whh", [128, NF, 4, 4, S4], f8, kind="ExternalInput")
    d_wih = nc.dram_tensor("wih", [16, 128, NF, S4], f8, kind="ExternalInput")
    d_wih0 = nc.dram_tensor("wih0", [128, 4 * S4], BF16, kind="ExternalInput")
    d_bias = nc.dram_tensor("bias", [128, 4 * S4], BF16, kind="ExternalInput")
    d_ev = nc.dram_tensor("ev", [1, cfg.T], BF16, kind="ExternalInput")
    d_vn = nc.dram_tensor("vnext", [128, cfg.NSLOT + 1], F32,
                          kind="ExternalInput")
    d_sel = nc.dram_tensor("sel", [128, 4], F32, kind="ExternalInput")
    d_eye = nc.dram_tensor("eye", [128, UB], BF16, kind="ExternalInput")
    d_ones = nc.dram_tensor("ones", [128, B], BF16, kind="ExternalInput")
    d_hout = nc.dram_tensor("hout", [128, NF], F32, kind="ExternalOutput")

    with ExitStack() as ctx:
        tc = ctx.enter_context(tile.TileContext(nc))
        const = ctx.enter_context(tc.tile_pool(name="const", bufs=1))
        state = ctx.enter_context(tc.tile_pool(name="state", bufs=1))
        evp = ctx.enter_context(tc.tile_pool(name="evp", bufs=2))
        wihp = ctx.enter_context(tc.tile_pool(name="wihp", bufs=3))
        tmp = ctx.enter_context(tc.tile_pool(name="tmp", bufs=2))
        agp = ctx.enter_context(tc.tile_pool(name="agp", bufs=1))
        xwgp = ctx.enter_context(tc.tile_pool(name="xwgp", bufs=2))
        psg = ctx.enter_context(tc.tile_pool(name="psg", bufs=1, space="PSUM"))
        psx = ctx.enter_context(tc.tile_pool(name="psx", bufs=2, space="PSUM"))
        dram = ctx.enter_context(tc.tile_pool(name="dram", bufs=1,
                                              space="DRAM"))

        # ---- resident constants ----
        whh = const.tile([128, NF, 4, 4, S4], f8, tag="whh")
        wih0 = const.tile([128, 4 * S4], BF16, tag="wih0")
        biast = const.tile([128, 4 * S4], BF16, tag="bias")
        vnt = const.tile([128, cfg.NSLOT + 1], F32, tag="vn")
        selt = const.tile([128, 4], F32, tag="sel")
        eye = const.tile([128, UB], BF16, tag="eye")
        ones = const.tile([128, B], BF16, tag="ones")
        for t_, d_ in [(whh, d_whh), (wih0, d_wih0), (biast, d_bias),
                       (vnt, d_vn), (selt, d_sel), (eye, d_eye),
                       (ones, d_ones)]:
            nc.sync.dma_start(out=t_, in_=d_[tuple(slice(None) for _ in
                                                   d_.shape)])

        # ---- persistent state ----
        hT = [state.tile([128, S4], BF16, tag=f"hT{i}", name=f"hT{i}")
              for i in range(2)]
        ct = [state.tile([128, S4], F32, tag=f"c{i}", name=f"c{i}")
              for i in range(2)]
        hblk = state.tile([128, NF, B], BF16, tag="hblk")
        hprev = [state.tile([128, NF, B], BF16, tag=f"hprev{i}",
                            name=f"hprev{i}") for i in range(2)]
        xw4 = [state.tile([128, 4 * S4], BF16, tag=f"xw4{i}", name=f"xw4{i}")
               for i in range(2)]
        ps = [psg.tile([128, S4], F32, tag=f"ps{x}", name=f"ps{x}")
              for x in range(4)]
        for t_ in hT + ct + [hblk] + hprev + ps:
            nc.vector.memset(t_, 0)

        # dram scratch (ping-pong xw blocks; UB pad rows for the prefetch
        # overrun on the last loop body)
        xwd = [dram.tile([B + UB, G], BF16, tag=f"xwd{i}", name=f"xwd{i}")
               for i in range(2)]
        agin = dram.tile([128, NF * B], BF16, tag="agin", name="agin")

        def gemm_xw(slot, xd):
            """xd (dram) <- masked xw block for the block consumed at
            slot+1 (layer l consumes block (slot+1) - SKEW*l)."""
            evs = min(max(slot + 1, 0), cfg.NBLK - 1) * B
            evb = evp.tile([128, B], BF16, tag="evb")
            for j in range(4):
                nc.sync.dma_start(out=evb[32 * j:32 * j + 1, :],
                                  in_=d_ev[0:1, evs:evs + B])
            hp = hprev[(slot + 1) % 2]
            for nt in range(16):
                j, x = nt // 4, nt % 4
                acc = psx.tile([B, S4], F32, tag="gacc")
                for qq in range(4):
                    wt = wihp.tile([128, NFQ, S4], f8, tag="wt")
                    nc.sync.dma_start(
                        out=wt, in_=d_wih[nt, :, qq * NFQ:(qq + 1) * NFQ, :])
                    for fq in range(NFQ):
                        fi = qq * NFQ + fq
                        nc.tensor.matmul(acc[:, :], hp[:, fi, :],
                                         wt[:, fq, :],
                                         start=(fi == 0), stop=False)
                nc.tensor.matmul(acc[:, :], evb[32 * j:32 * j + 1, :],
                                 wih0[32 * j:32 * j + 1,
                                      x * S4:(x + 1) * S4],
                                 start=False, stop=False,
                                 tile_position=(32 * j, 0))
                nc.tensor.matmul(acc[:, :], ones[32 * j:32 * j + 1, :],
                                 biast[32 * j:32 * j + 1,
                                       x * S4:(x + 1) * S4],
                                 start=False, stop=True,
                                 tile_position=(32 * j, 0))
                xwg = xwgp.tile([B, S4], BF16, tag="xwg")
                nc.vector.tensor_scalar_mul(xwg[:, :], acc[:, :],
                                            vnt[0:B, slot + 1:slot + 2])
                nc.sync.dma_start(out=xd[0:B, nt * S4:(nt + 1) * S4],
                                  in_=xwg[:, :])

        def step(xwt, u, copy_h):
            pin, pout = u % 2, 1 - (u % 2)
            for x in range(4):
                for j in range(4):
                    nc.tensor.matmul(
                        ps[x][32 * j:32 * j + 1, :],
                        eye[32 * j:32 * j + UB, u:u + 1],
                        xwt[32 * j:32 * j + UB, x * S4:(x + 1) * S4],
                        start=True, stop=False,
                        tile_position=(32 * j, 32 * j))
                for kc in range(NF):
                    for j in range(4):
                        nc.tensor.matmul(
                            ps[x][32 * j:32 * j + 1, :],
                            hT[pin][:, 32 * kc:32 * kc + 1],
                            whh[:, kc, j, x, :],
                            start=False, stop=(kc == NF - 1),
                            tile_position=(0, 32 * j))
            si = tmp.tile([128, S4], F32, tag="si")
            sf = tmp.tile([128, S4], BF16, tag="sf")
            tg = tmp.tile([128, S4], BF16, tag="tg")
            so = tmp.tile([128, S4], BF16, tag="so")
            nc.scalar.activation(si, ps[0][:, :], Sig, scale=inv)
            nc.scalar.activation(sf, ps[1][:, :], Sig, scale=inv)
            nc.scalar.activation(tg, ps[2][:, :], Tanh, scale=inv)
            nc.scalar.activation(so, ps[3][:, :], Sig, scale=inv)
            nc.vector.tensor_mul(si[:, :], si[:, :], tg[:, :])
            nc.vector.tensor_mul(ct[pout][:, :], sf[:, :], ct[pin][:, :])
            nc.vector.tensor_add(ct[pout][:, :], ct[pout][:, :], si[:, :])
            nc.scalar.activation(tg, ct[pout][:, :], Tanh)
            hh = tmp.tile([128, S4], BF16, tag="hh")
            nc.vector.tensor_mul(hh, so[:, :], tg[:, :])
            nc.vector.transpose(hT[pout][:, :], hh[:, :])
            copy_h(hT[pout])

        def load_xw4(dst, xd, rows):
            """dst[32j:32j+UB, :] <- xd[rows, quarter j] for each quarter."""
            for j in range(4):
                nc.sync.dma_start(
                    out=dst[32 * j:32 * j + UB, :],
                    in_=xd[rows, 4 * j * S4:(4 * j + 4) * S4])

        def recur_slot(s):
            xd = xwd[s % 2]
            # prologue: load first iteration's xw4 into slot A
            load_xw4(xw4[0], xd, slice(0, UB))
            def mk_copy(t_expr):
                def copy_h(ht):
                    nc.vector.tensor_copy(
                        out=hblk[:, :, t_expr],
                        in_=ht[:].rearrange(
                            "p (a b) -> p a b", b=32)[:, :, 0:1])
                return copy_h

            with tc.For_i(0, cfg.NIT // 2, 1,
                          hint_engines=(mybir.EngineType.PE,)) as iv:
                # prefetch iteration 2iv+1 into B while computing on A
                load_xw4(xw4[1], xd, bass.ds((2 * iv + 1) * UB, UB))
                for u in range(UB):
                    step(xw4[0], u,
                         mk_copy(bass.ds((2 * iv) * UB + u, 1)))
                # prefetch iteration 2iv+2 into A while computing on B
                load_xw4(xw4[0], xd, bass.ds((2 * iv + 2) * UB, UB))
                for u in range(UB):
                    step(xw4[1], u,
                         mk_copy(bass.ds((2 * iv + 1) * UB + u, 1)))

        def ag_slot(s):
            agout = dram.tile([cfg.NCORES * 128, NF * B], BF16,
                              tag=f"agout{s}", addr_space="Shared",
                              name=f"agout{s}")
            nc.sync.dma_start(out=agin[:, :],
                              in_=hblk[:].rearrange("p a b -> p (a b)"))
            nc.gpsimd.collective_compute(
                "AllGather", mybir.AluOpType.bypass,
                replica_groups=[list(range(cfg.NCORES))],
                ins=[agin[:].opt()], outs=[agout[:].opt()])
            return agout

        def combine(s, agout):
            """hprev[(s+1)%2] <- masked sum of the 4 layer-core h blocks
            from AG(s-? ) output."""
            hpf = hprev[(s + 1) % 2][:].rearrange("p a b -> p (a b)")
            a2 = agp.tile([128, NF * B], BF16, tag="agt2")
            for r in range(4):
                at = agp.tile([128, NF * B], BF16, tag="agt", name=f"agt{r}")
                nc.sync.dma_start(out=at,
                                  in_=agout[128 * r:128 * (r + 1), :])
                if r == 0:
                    nc.vector.tensor_scalar_mul(hpf, at[:, :],
                                                selt[:, 0:1])
                else:
                    nc.vector.tensor_scalar_mul(a2[:, :], at[:, :],
                                                selt[:, r:r + 1])
                    nc.vector.tensor_add(hpf, hpf, a2[:, :])

        # ---------------- program ----------------
        gemm_xw(-1, xwd[0])
        agouts = {}
        for s in range(cfg.NSLOT):
            recur_slot(s)
            if s <= cfg.NSLOT - 2:
                agouts[s] = ag_slot(s)
            if s <= cfg.NSLOT - 2:
                if s >= 1:
                    combine(s, agouts[s - 1])
                gemm_xw(s, xwd[(s + 1) % 2])

        hout = const.tile([128, NF], F32, tag="hout")
        nc.vector.tensor_copy(out=hout, in_=hblk[:, :, B - 1])
        nc.sync.dma_start(out=d_hout[:, :], in_=hout[:, :])

    nc.compile()
    return nc


def unpermute_h(cfg, hout):
    """hout [128, NF] -> h [H2] (undo the stationary permutation)."""
    perm = perm_cols(cfg)                    # [NF, 128]
    h = np.zeros(cfg.H2, np.float32)
    h[perm.T.reshape(-1)] = np.asarray(hout, np.float32).reshape(-1)
    return h


def head(h, w_out, b_out):
    logits = h @ np.asarray(w_out, np.float32).T + np.asarray(b_out,
                                                              np.float32)
    m = logits.max()
    out = logits - (np.log(np.exp(logits - m).sum()) + m)
    return out[None, :].astype(np.float32)


_BUILD_CACHE = {}


def kernel(event, w_ih0, w_ih, w_hh, b_ih, b_hh, w_out, b_out):
    from concourse.bass_utils import run_bass_kernel_spmd

    cfg = Cfg()
    event = np.asarray(event, np.float32)
    in_maps = [prep_core_inputs(cfg, c, event, np.asarray(w_ih0, np.float32),
                                np.asarray(w_ih, np.float32),
                                np.asarray(w_hh, np.float32),
                                np.asarray(b_ih, np.float32),
                                np.asarray(b_hh, np.float32))
               for c in range(cfg.NCORES)]
    key = "full"
    if key not in _BUILD_CACHE:
        _BUILD_CACHE[key] = build(cfg)
    nc = _BUILD_CACHE[key]
    res = run_bass_kernel_spmd(nc, in_maps, core_ids=list(range(cfg.NCORES)))
    hout = res.results[cfg.L - 1]["hout"]
    h = unpermute_h(cfg, hout)
    return head(h, w_out, b_out)


# revision 23
# speedup vs baseline: 1.0374x; 1.0374x over previous
"""Trainium2 Bass kernel for nn_AwkwardRNN (4-layer LSTM, H2=2048, T=2048, batch-1).

Design ("layer-per-core wavefront pipeline", v2):
  - Cores 0-3 each own one LSTM layer. W_hh lives in SBUF as fp8 (pre-scaled
    by SCALE so values sit in e4m3's normal range); h is bf16; PSUM
    accumulates fp32; cell state c is fp32. Cores 4-7 run the same program
    on zero weights (outputs ignored).
  - The sequence runs in blocks of B=64 steps with a per-layer slot skew of
    2 (layer l processes block b in slot b+2l). The extra slot of skew gives
    the inter-layer AllGather a full slot of slack: AG(s) is issued at the
    end of slot s and consumed by the xw GEMM at the end of slot s+1, so the
    collective latency never stalls the PE.
  - Per step, W_hh @ h streams W as the matmul moving operand (h is a
    one-column stationary), 4-way column-tiled. The per-step xw[t] is
    injected as an extra K=UB one-hot matmul chunk.
  - xw blocks live in DRAM (ping-pong); the per-UB-iteration xw4 SBUF tile
    is double-buffered with a one-iteration prefetch distance inside the
    hardware loop (body covers 2 iterations = 8 steps), so the DMA latency
    is fully hidden.
  - h is re-laid-out for the next step's stationary with a DVE 32x32 block
    transpose; a host-side weight permutation absorbs the layout.
"""

import sys

for _p in ("/opt/trn_rl_repo",):
    if _p not in sys.path:
        sys.path.insert(0, _p)

from contextlib import ExitStack

import numpy as np
import ml_dtypes

import concourse.bacc as bacc
import concourse.bass as bass
import concourse.tile as tile
from concourse import mybir

F32 = mybir.dt.float32
BF16 = mybir.dt.bfloat16


class Cfg:
    def __init__(self, H2=2048, T=2048, B=64, UB=4, L=4, NCORES=8,
                 SCALE=1024.0, SKEW=2):
        self.H2, self.T, self.B, self.UB, self.L = H2, T, B, UB, L
        self.NCORES, self.SCALE, self.SKEW = NCORES, SCALE, SKEW
        self.G = 4 * H2
        self.S4 = H2 // 4           # hidden slice per col-group
        self.NF = H2 // 128         # stationary h chunks
        self.NBLK = T // B
        self.NSLOT = self.NBLK + SKEW * (L - 1)
        self.NIT = B // UB
        assert H2 % 128 == 0 and T % B == 0 and B % UB == 0 and UB % 2 == 0
        assert self.S4 % 32 == 0 and self.NF % 4 == 0 and self.NIT % 2 == 0

    @property
    def W_DT(self):
        return mybir.dt.float8e4

    @property
    def W_NP(self):
        return ml_dtypes.float8_e4m3


def perm_cols(cfg):
    """perm[fi, p] = hidden index held at (partition p, stationary chunk fi)."""
    fi = np.arange(cfg.NF)[:, None]
    p = np.arange(128)[None, :]
    return cfg.S4 * (p // 32) + 32 * fi + (p % 32)


def gate_order(cfg):
    """gidx[nt*S4 + q] = weight row of xw column (nt=(j*4+x), q)."""
    H2, S4 = cfg.H2, cfg.S4
    gidx = np.zeros(cfg.G, np.int64)
    for j in range(4):
        for x in range(4):
            nt = j * 4 + x
            gidx[nt * S4:(nt + 1) * S4] = x * H2 + S4 * j + np.arange(S4)
    return gidx


def _eye_rep(cfg):
    e = np.zeros((128, cfg.UB), ml_dtypes.bfloat16)
    for j in range(4):
        for u in range(cfg.UB):
            e[32 * j + u, u] = 1
    return e


def pack_rows(cfg, vec):
    """[G] gate-ordered vector -> [128, 4*S4] with row 32j = (j,*) slices."""
    out = np.zeros((128, 4 * cfg.S4), vec.dtype)
    for j in range(4):
        out[32 * j] = vec[4 * j * cfg.S4:(4 * j + 4) * cfg.S4]
    return out


def prep_core_inputs(cfg, core, event, w_ih0, w_ih, w_hh, b_ih, b_hh):
    H2, S4, NF, B, G = cfg.H2, cfg.S4, cfg.NF, cfg.B, cfg.G
    perm = perm_cols(cfg)
    gidx = gate_order(cfg)
    lay = core if core < cfg.L else None
    bf = ml_dtypes.bfloat16

    whh = np.zeros((128, NF, 4, 4, S4), cfg.W_NP)
    if lay is not None:
        W = (w_hh[lay] * cfg.SCALE).astype(np.float32)
        q = np.arange(S4)
        for kc in range(NF):
            Wc = W[:, perm[kc]]                     # [G, 128]
            for j in range(4):
                for x in range(4):
                    rows = x * H2 + S4 * j + q
                    whh[:, kc, j, x, :] = Wc[rows, :].T.astype(cfg.W_NP)

    # wih layout: [16(nt), 128(p), NF(fi), S4(q)]
    wih = np.zeros((16, 128, NF, S4), cfg.W_NP)
    if lay is not None and lay > 0:
        W = (w_ih[lay - 1] * cfg.SCALE).astype(np.float32)
        q = np.arange(S4)
        for fi in range(NF):
            Wc = W[:, perm[fi]]                     # [G, 128]
            for j in range(4):
                for x in range(4):
                    nt = j * 4 + x
                    rows = x * H2 + S4 * j + q
                    wih[nt, :, fi, :] = Wc[rows, :].T.astype(cfg.W_NP)

    g0 = np.zeros(G, np.float32)
    if lay == 0:
        g0 = (w_ih0[:, 0] * cfg.SCALE)[gidx]
    wih0 = pack_rows(cfg, g0).astype(bf)

    gb = np.zeros(G, np.float32)
    if lay is not None:
        gb = ((b_ih[lay] + b_hh[lay]) * cfg.SCALE)[gidx]
    bias = pack_rows(cfg, gb).astype(bf)

    ev = np.zeros((1, cfg.T), bf)
    if lay == 0:
        ev[0] = event.astype(bf)

    # gmask[:, i] = 1 if block (i - SKEW*l) is valid for this core's layer.
    # gemm for slot s (producing xw consumed in slot s+1) masks with column
    # s+1; the prologue gemm (s=-1) uses column 0.
    gmask = np.zeros((128, cfg.NSLOT + 1), np.float32)
    if lay is not None:
        for i in range(cfg.NSLOT + 1):
            b = i - cfg.SKEW * lay
            gmask[:, i] = 1.0 if 0 <= b < cfg.NBLK else 0.0

    sel = np.zeros((128, 4), np.float32)
    if lay is not None and lay > 0:
        sel[:, lay - 1] = 1.0

    return {
        "whh": whh, "wih": wih, "wih0": wih0, "bias": bias, "ev": ev,
        "vnext": gmask, "sel": sel,
        "eye": _eye_rep(cfg),
        "ones": np.ones((128, B), bf),
    }


def build(cfg):
    H2, S4, NF, B, G, UB = cfg.H2, cfg.S4, cfg.NF, cfg.B, cfg.G, cfg.UB
    f8 = cfg.W_DT
    Sig = mybir.ActivationFunctionType.Sigmoid
    Tanh = mybir.ActivationFunctionType.Tanh
    inv = 1.0 / cfg.SCALE
    NFQ = NF // 4

    nc = bacc.Bacc("TRN2", target_bir_lowering=False)

    d_whh = nc.dram_tensor("whh", [128, NF, 4, 4, S4], f8, kind="ExternalInput")
    d_wih = nc.dram_tensor("wih", [16, 128, NF, S4], f8, kind="ExternalInput")
    d_wih0 = nc.dram_tensor("wih0", [128, 4 * S4], BF16, kind="ExternalInput")
    d_bias = nc.dram_tensor("bias", [128, 4 * S4], BF16, kind="ExternalInput")
    d_ev = nc.dram_tensor("ev", [1, cfg.T], BF16, kind="ExternalInput")
    d_vn = nc.dram_tensor("vnext", [128, cfg.NSLOT + 1], F32,
                          kind="ExternalInput")
    d_sel = nc.dram_tensor("sel", [128, 4], F32, kind="ExternalInput")
    d_eye = nc.dram_tensor("eye", [128, UB], BF16, kind="ExternalInput")
    d_ones = nc.dram_tensor("ones", [128, B], BF16, kind="ExternalInput")
    d_hout = nc.dram_tensor("hout", [128, NF], F32, kind="ExternalOutput")

    with ExitStack() as ctx:
        tc = ctx.enter_context(tile.TileContext(nc))
        const = ctx.enter_context(tc.tile_pool(name="const", bufs=1))
        state = ctx.enter_context(tc.tile_pool(name="state", bufs=1))
        evp = ctx.enter_context(tc.tile_pool(name="evp", bufs=2))
        wihp = ctx.enter_context(tc.tile_pool(name="wihp", bufs=4))
        tmp = ctx.enter_context(tc.tile_pool(name="tmp", bufs=2))
        agp = ctx.enter_context(tc.tile_pool(name="agp", bufs=1))
        xwgp = ctx.enter_context(tc.tile_pool(name="xwgp", bufs=2))
        psg = ctx.enter_context(tc.tile_pool(name="psg", bufs=1, space="PSUM"))
        psx = ctx.enter_context(tc.tile_pool(name="psx", bufs=2, space="PSUM"))
        dram = ctx.enter_context(tc.tile_pool(name="dram", bufs=1,
                                              space="DRAM"))

        # ---- resident constants ----
        whh = const.tile([128, NF, 4, 4, S4], f8, tag="whh")
        wih0 = const.tile([128, 4 * S4], BF16, tag="wih0")
        biast = const.tile([128, 4 * S4], BF16, tag="bias")
        vnt = const.tile([128, cfg.NSLOT + 1], F32, tag="vn")
        selt = const.tile([128, 4], F32, tag="sel")
        eye = const.tile([128, UB], BF16, tag="eye")
        ones = const.tile([128, B], BF16, tag="ones")
        for t_, d_ in [(whh, d_whh), (wih0, d_wih0), (biast, d_bias),
                       (vnt, d_vn), (selt, d_sel), (eye, d_eye),
                       (ones, d_ones)]:
            nc.sync.dma_start(out=t_, in_=d_[tuple(slice(None) for _ in
                                                   d_.shape)])

        # ---- persistent state ----
        hT = [state.tile([128, S4], BF16, tag=f"hT{i}", name=f"hT{i}")
              for i in range(2)]
        ct = [state.tile([128, S4], F32, tag=f"c{i}", name=f"c{i}")
              for i in range(2)]
        hblk = state.tile([128, NF, B], BF16, tag="hblk")
        hprev = [state.tile([128, NF, B], BF16, tag=f"hprev{i}",
                            name=f"hprev{i}") for i in range(2)]
        xw4 = [state.tile([128, 4 * S4], BF16, tag=f"xw4{i}", name=f"xw4{i}")
               for i in range(2)]
        ps = [psg.tile([128, S4], F32, tag=f"ps{x}", name=f"ps{x}")
              for x in range(4)]
        for t_ in hT + ct + [hblk] + hprev + ps:
            nc.vector.memset(t_, 0)

        # dram scratch (ping-pong xw blocks; UB pad rows for the prefetch
        # overrun on the last loop body)
        xwd = [dram.tile([B + UB, G], BF16, tag=f"xwd{i}", name=f"xwd{i}")
               for i in range(2)]
        agin = dram.tile([128, NF * B], BF16, tag="agin", name="agin")

        def gemm_xw(slot, xd):
            """xd (dram) <- masked xw block for the block consumed at
            slot+1 (layer l consumes block (slot+1) - SKEW*l)."""
            evs = min(max(slot + 1, 0), cfg.NBLK - 1) * B
            evb = evp.tile([128, B], BF16, tag="evb")
            for j in range(4):
                nc.sync.dma_start(out=evb[32 * j:32 * j + 1, :],
                                  in_=d_ev[0:1, evs:evs + B])
            hp = hprev[(slot + 1) % 2]
            for nt in range(16):
                j, x = nt // 4, nt % 4
                acc = psx.tile([B, S4], F32, tag="gacc")
                for qq in range(4):
                    wt = wihp.tile([128, NFQ, S4], f8, tag="wt")
                    nc.sync.dma_start(
                        out=wt, in_=d_wih[nt, :, qq * NFQ:(qq + 1) * NFQ, :])
                    for fq in range(NFQ):
                        fi = qq * NFQ + fq
                        nc.tensor.matmul(acc[:, :], hp[:, fi, :],
                                         wt[:, fq, :],
                                         start=(fi == 0), stop=False)
                nc.tensor.matmul(acc[:, :], evb[32 * j:32 * j + 1, :],
                                 wih0[32 * j:32 * j + 1,
                                      x * S4:(x + 1) * S4],
                                 start=False, stop=False,
                                 tile_position=(32 * j, 0))
                nc.tensor.matmul(acc[:, :], ones[32 * j:32 * j + 1, :],
                                 biast[32 * j:32 * j + 1,
                                       x * S4:(x + 1) * S4],
                                 start=False, stop=True,
                                 tile_position=(32 * j, 0))
                xwg = xwgp.tile([B, S4], BF16, tag="xwg")
                nc.vector.tensor_scalar_mul(xwg[:, :], acc[:, :],
                                            vnt[0:B, slot + 1:slot + 2])
                nc.sync.dma_start(out=xd[0:B, nt * S4:(nt + 1) * S4],
                                  in_=xwg[:, :])

        def step(xwt, u, copy_h):
            pin, pout = u % 2, 1 - (u % 2)
            for x in range(4):
                for j in range(4):
                    nc.tensor.matmul(
                        ps[x][32 * j:32 * j + 1, :],
                        eye[32 * j:32 * j + UB, u:u + 1],
                        xwt[32 * j:32 * j + UB, x * S4:(x + 1) * S4],
                        start=True, stop=False,
                        tile_position=(32 * j, 32 * j))
                for kc in range(NF):
                    for j in range(4):
                        nc.tensor.matmul(
                            ps[x][32 * j:32 * j + 1, :],
                            hT[pin][:, 32 * kc:32 * kc + 1],
                            whh[:, kc, j, x, :],
                            start=False, stop=(kc == NF - 1),
                            tile_position=(0, 32 * j))
            si = tmp.tile([128, S4], F32, tag="si")
            sf = tmp.tile([128, S4], BF16, tag="sf")
            tg = tmp.tile([128, S4], BF16, tag="tg")
            so = tmp.tile([128, S4], BF16, tag="so")
            nc.scalar.activation(si, ps[0][:, :], Sig, scale=inv)
            nc.scalar.activation(sf, ps[1][:, :], Sig, scale=inv)
            nc.scalar.activation(tg, ps[2][:, :], Tanh, scale=inv)
            nc.scalar.activation(so, ps[3][:, :], Sig, scale=inv)
            nc.vector.tensor_mul(si[:, :], si[:, :], tg[:, :])
            nc.vector.tensor_mul(ct[pout][:, :], sf[:, :], ct[pin][:, :])
            nc.vector.tensor_add(ct[pout][:, :], ct[pout][:, :], si[:, :])
            nc.scalar.activation(tg, ct[pout][:, :], Tanh)
            hh = tmp.tile([128, S4], BF16, tag="hh")
            nc.vector.tensor_mul(hh, so[:, :], tg[:, :])
            nc.vector.transpose(hT[pout][:, :], hh[:, :])
            copy_h(hT[pout])

        def load_xw4(dst, xd, rows):
            """dst[32j:32j+UB, :] <- xd[rows, quarter j] for each quarter."""
            for j in range(4):
                nc.sync.dma_start(
                    out=dst[32 * j:32 * j + UB, :],
                    in_=xd[rows, 4 * j * S4:(4 * j + 4) * S4])

        def recur_slot(s):
            xd = xwd[s % 2]
            # prologue: load first iteration's xw4 into slot A
            load_xw4(xw4[0], xd, slice(0, UB))
            def mk_copy(t_expr):
                def copy_h(ht):
                    nc.vector.tensor_copy(
                        out=hblk[:, :, t_expr],
                        in_=ht[:].rearrange(
                            "p (a b) -> p a b", b=32)[:, :, 0:1])
                return copy_h

            with tc.For_i(0, cfg.NIT // 2, 1,
                          hint_engines=(mybir.EngineType.PE,)) as iv:
                # prefetch iteration 2iv+1 into B while computing on A
                load_xw4(xw4[1], xd, bass.ds((2 * iv + 1) * UB, UB))
                for u in range(UB):
                    step(xw4[0], u,
                         mk_copy(bass.ds((2 * iv) * UB + u, 1)))
                # prefetch iteration 2iv+2 into A while computing on B
                load_xw4(xw4[0], xd, bass.ds((2 * iv + 2) * UB, UB))
                for u in range(UB):
                    step(xw4[1], u,
                         mk_copy(bass.ds((2 * iv + 1) * UB + u, 1)))

        def ag_slot(s):
            agout = dram.tile([cfg.NCORES * 128, NF * B], BF16,
                              tag=f"agout{s}", addr_space="Shared",
                              name=f"agout{s}")
            nc.sync.dma_start(out=agin[:, :],
                              in_=hblk[:].rearrange("p a b -> p (a b)"))
            nc.gpsimd.collective_compute(
                "AllGather", mybir.AluOpType.bypass,
                replica_groups=[list(range(cfg.NCORES))],
                ins=[agin[:].opt()], outs=[agout[:].opt()])
            return agout

        def combine(s, agout):
            """hprev[(s+1)%2] <- masked sum of the 4 layer-core h blocks
            from AG(s-? ) output."""
            hpf = hprev[(s + 1) % 2][:].rearrange("p a b -> p (a b)")
            a2 = agp.tile([128, NF * B], BF16, tag="agt2")
            for r in range(4):
                at = agp.tile([128, NF * B], BF16, tag="agt", name=f"agt{r}")
                nc.sync.dma_start(out=at,
                                  in_=agout[128 * r:128 * (r + 1), :])
                if r == 0:
                    nc.vector.tensor_scalar_mul(hpf, at[:, :],
                                                selt[:, 0:1])
                else:
                    nc.vector.tensor_scalar_mul(a2[:, :], at[:, :],
                                                selt[:, r:r + 1])
                    nc.vector.tensor_add(hpf, hpf, a2[:, :])

        # ---------------- program ----------------
        gemm_xw(-1, xwd[0])
        agouts = {}
        for s in range(cfg.NSLOT):
            recur_slot(s)
            if s <= cfg.NSLOT - 2:
                agouts[s] = ag_slot(s)
            if s <= cfg.NSLOT - 2:
                if s >= 1:
                    combine(s, agouts[s - 1])
                gemm_xw(s, xwd[(s + 1) % 2])

        hout = const.tile([128, NF], F32, tag="hout")
        nc.vector.tensor_copy(out=hout, in_=hblk[:, :, B - 1])
        nc.sync.dma_start(out=d_hout[:, :], in_=hout[:, :])

    nc.compile()
    return nc


def unpermute_h(cfg, hout):
    """hout [128, NF] -> h [H2] (undo the stationary permutation)."""
    perm = perm_cols(cfg)                    # [NF, 128]
    h = np.zeros(cfg.H2, np.float32)
    h[perm.T.reshape(-1)] = np.asarray(hout, np.float32).reshape(-1)
    return h


def head(h, w_out, b_out):
    logits = h @ np.asarray(w_out, np.float32).T + np.asarray(b_out,
                                                              np.float32)
    m = logits.max()
    out = logits - (np.log(np.exp(logits - m).sum()) + m)
    return out[None, :].astype(np.float32)


_BUILD_CACHE = {}


def kernel(event, w_ih0, w_ih, w_hh, b_ih, b_hh, w_out, b_out):
    from concourse.bass_utils import run_bass_kernel_spmd

    cfg = Cfg()
    event = np.asarray(event, np.float32)
    in_maps = [prep_core_inputs(cfg, c, event, np.asarray(w_ih0, np.float32),
                                np.asarray(w_ih, np.float32),
                                np.asarray(w_hh, np.float32),
                                np.asarray(b_ih, np.float32),
                                np.asarray(b_hh, np.float32))
               for c in range(cfg.NCORES)]
    key = "full"
    if key not in _BUILD_CACHE:
        _BUILD_CACHE[key] = build(cfg)
    nc = _BUILD_CACHE[key]
    res = run_bass_kernel_spmd(nc, in_maps, core_ids=list(range(cfg.NCORES)))
    hout = res.results[cfg.L - 1]["hout"]
    h = unpermute_h(cfg, hout)
    return head(h, w_out, b_out)
